# revision 16
# baseline (speedup 1.0000x reference)
"""GATNet Trainium kernel v2: host preprocessing + Bass program builder.

Design (8-way dst-shard of nodes, bf16 throughout):
  node phase (per-shard): npsum = xT_chunk.T @ [Wh0|Wh1|was|wad] (PE bf16)
    -> staged into 256-col T rows [h0|1|0|h1|1|0|es(f32 @204:208)|pad] + ed_tab (bf16)
  exchange: AllGather of Tshard [6272, 256] bf16 -> Tfull (ping-pong A/B)
  aggregation (per-shard dst windows of 128 dsts):
    - ONE dma_gather per window half (int16 idx limit splits rows at 32768)
      pulls all ~11*128 edge rows (512B each) in 2 SWDGE instructions
    - edp = stt_j^T @ ed (PE, one-hot from DRAM stblob bf16)
    - e = es_src + ed_dst (DVE), w = Exp(Prelu(e)) (ACT)
    - in-place Vt: Gt[h|1] *= w per head (DVE broadcast mul)
    - segment-sum: apsum += st_j^T @ Gt_j[0:203] (PE, one j-matmul per tile)
    - epilogue: ACT Relu(apsum * 1/s) per head -> stb bf16; PE transpose ->
      persistent SBUF xT buffers (no DRAM roundtrip)
  pool phase fused into last agg layer: ppA/ppB += stb^T @ pmask(rcnt-baked),
  AllReduce [203, 256] f32, 3-layer MLP on PE/ACT.
"""
import sys
sys.path.insert(0, "/opt/trn_rl_repo")
sys.path.insert(0, "/opt/trn_rl_repo/concourse")
import numpy as np
import ml_dtypes
from concourse import bass, bacc, mybir
import concourse.tile as tile

BF16 = mybir.dt.bfloat16
F32 = mybir.dt.float32
I16 = mybir.dt.int16
AF = mybir.ActivationFunctionType
ALU = mybir.AluOpType
BF = ml_dtypes.bfloat16

N = 50000
E = 400000
G = 256
H = 2
C = 100
F_IN = 336
SLOPE = 0.2
NCORE = 8
SHARD = N // NCORE            # 6250
P = 128
SHARD_PAD = 6272              # 49*128
NW = 49
NT = 49
NFULL_PAD = NCORE * SHARD_PAD # 50176
RHALF = 3200                  # rows of chunk A (node tiles 0:24)
CHA = NCORE * RHALF           # 25600 rows of chunk-A block in Tfull
SPLIT = 32768                 # int16 idx limit for dma_gather
EL = 256                      # T row cols (bf16) = 512B
SCOLS = 203                   # stb cols: [x0(100)|s0|z|x1(100)|s1]


def _wrap16(vals):
    """int16 idx list -> [128, ceil(n/16)] wrapped layout (idx i at [i%16, i//16])."""
    n = len(vals)
    k = -(-n // 16)
    arr = np.zeros((16, k), np.int16)
    arr[np.arange(n) % 16, np.arange(n) // 16] = vals
    return np.tile(arr, (8, 1))


def prep(x, edge_index, batch, Ws, asrcs, adsts, bcs, lws, lbs):
    src_all = np.concatenate([edge_index[0], np.arange(N, dtype=np.int64)])
    dst_all = np.concatenate([edge_index[1], np.arange(N, dtype=np.int64)])
    order = np.argsort(dst_all, kind="stable")
    s_sorted = src_all[order].astype(np.int64)
    d_sorted = dst_all[order].astype(np.int64)
    s_k = s_sorted // SHARD
    s_r = s_sorted % SHARD
    s_row = np.where(s_r < RHALF, s_k * RHALF + s_r,
                     CHA + s_k * (SHARD_PAD - RHALF) + (s_r - RHALF))

    # per (core, window) edge ranges + lo/hi split sizes
    win = {}
    q1_req = np.zeros((NCORE, NW), np.int64)
    q2_req = np.zeros((NCORE, NW), np.int64)
    for k in range(NCORE):
        for w in range(NW):
            d_lo = k * SHARD + w * P
            d_hi = min(k * SHARD + (w + 1) * P, (k + 1) * SHARD)
            lo = np.searchsorted(d_sorted, d_lo, side="left")
            hi = np.searchsorted(d_sorted, d_hi, side="left")
            sr = s_row[lo:hi]
            m = sr < SPLIT
            win[(k, w)] = (lo, hi, m)
            q1_req[k, w] = int(m.sum())
            q2_req[k, w] = int((~m).sum())
    n1max = q1_req.max(axis=0)
    n2max = q2_req.max(axis=0)
    Q1w = -(-n1max // P)
    Q2w = -(-n2max // P)
    SMw = Q1w + Q2w
    NJ = int(SMw.sum())

    per_core = []
    for k in range(NCORE):
        idxb = np.zeros((P, 8 * NJ), np.int16)
        dlocb = np.full((P, NJ), -1.0, np.float32)
        sb = np.zeros((P, NJ * P), BF)
        offj = 0
        for w in range(NW):
            lo, hi, m = win[(k, w)]
            Q1, Q2 = int(Q1w[w]), int(Q2w[w])
            SM = Q1 + Q2
            sr = s_row[lo:hi]
            dl = d_sorted[lo:hi] - (k * SHARD + w * P)
            n1 = int(m.sum())
            n2 = int((~m).sum())
            olo = np.argsort(sr[m], kind="stable")
            ohi = np.argsort(sr[~m], kind="stable")
            ilo = np.zeros(Q1 * P, np.int64)
            ilo[:n1] = sr[m][olo]
            ihi = np.zeros(Q2 * P, np.int64)
            ihi[:n2] = sr[~m][ohi] - SPLIT
            wrapped = np.concatenate(
                [_wrap16(ilo.astype(np.int16)), _wrap16(ihi.astype(np.int16))]
                if Q2 > 0 else [_wrap16(ilo.astype(np.int16))], axis=1)
            idxb[:, 8 * offj:8 * (offj + SM)] = wrapped
            dslot = np.full(SM * P, -1, np.int64)
            dslot[:n1] = dl[m][olo]
            dslot[Q1 * P:Q1 * P + n2] = dl[~m][ohi]
            dlocb[:, offj:offj + SM] = dslot.reshape(SM, P).T.astype(np.float32)
            # transposed one-hot stt_j[mm, p] for the ed matmul
            oh = np.zeros((SM * P, P), np.float32)
            valid = dslot >= 0
            oh[np.arange(SM * P)[valid], dslot[valid]] = 1.0
            oh3 = oh.reshape(SM, P, P)
            sb[:, offj * P:(offj + SM) * P] = (
                oh3.transpose(2, 0, 1).reshape(P, SM * P).astype(BF))
            offj += SM
        xT0 = np.zeros((384, SHARD_PAD), BF)
        xT0[:F_IN, :SHARD] = x[k * SHARD:(k + 1) * SHARD].T.astype(BF)
        # pool mask with 1/cnt baked in
        cnt = np.bincount(batch, minlength=G).astype(np.float32)
        rcnt = 1.0 / np.maximum(cnt, 1.0)
        pmask = np.zeros((NT, P, G), BF)
        bsh = batch[k * SHARD:(k + 1) * SHARD]
        for t in range(NT):
            r0, r1 = t * P, min(t * P + P, SHARD)
            if r1 > r0:
                pmask[t, np.arange(r1 - r0), bsh[r0:r1]] = rcnt[bsh[r0:r1]].astype(BF)
        per_core.append(dict(xT0=xT0, idxb=idxb, dlocb=dlocb, sblob=sb,
                             pmask=pmask))

    # weights: npsum cols [h0(100)|h1(100)|es0 es1|ed0 ed1] = 204
    def make_aug(W, a_s, a_d):
        F = W.shape[0]
        was = np.zeros((F, 2), np.float32)
        wad = np.zeros((F, 2), np.float32)
        for h in range(H):
            was[:, h] = W[:, h * C:(h + 1) * C] @ a_s[h]
            wad[:, h] = W[:, h * C:(h + 1) * C] @ a_d[h]
        return np.concatenate(
            [W[:, 0:C], W[:, C:2 * C], was, wad], axis=1)  # [F, 204]

    # L0: 3 chunks of rows (336 padded to 384)
    waug = np.zeros((11, P, 204), BF)
    aug0 = make_aug(Ws[0].astype(np.float64), asrcs[0], adsts[0]).astype(np.float32)
    for c in range(3):
        rows = aug0[c * P:min((c + 1) * P, F_IN)]
        waug[c, :rows.shape[0]] = rows.astype(BF)
    # L1-4: features live at stb cols (f<100 -> col f; f>=100 -> col f+2)
    for li in range(1, 5):
        aug = make_aug(Ws[li].astype(np.float64), asrcs[li], adsts[li]).astype(np.float32)
        perm = np.zeros((SCOLS, 204), np.float32)
        perm[0:100] = aug[0:100]
        perm[102:202] = aug[100:200]
        waug[3 + 2 * (li - 1), :, :] = perm[0:128].astype(BF)
        waug[4 + 2 * (li - 1), 0:SCOLS - 128, :] = perm[128:SCOLS].astype(BF)

    # MLP weights with same row-perm for lw1
    w1p = np.zeros((SCOLS, 100), np.float32)
    w1p[0:100] = lws[0][0:100]
    w1p[102:202] = lws[0][100:200]
    mlw1a = w1p[0:128].astype(np.float32)
    mlw1b = np.zeros((P, 100), np.float32)
    mlw1b[0:SCOLS - 128] = w1p[128:SCOLS]
    iota = np.broadcast_to(np.arange(P, dtype=np.float32), (P, P)).astype(BF)
    ident = np.eye(P, dtype=BF)

    has_bias = any(np.abs(b).max() > 0 for b in bcs) or any(
        np.abs(b).max() > 0 for b in lbs)
    biasrep = np.zeros((5, P, SCOLS), BF)
    for li in range(5):
        biasrep[li, :, 0:100] = bcs[li][0:100]
        biasrep[li, :, 102:202] = bcs[li][100:200]
    mlpb = np.zeros((3, P, 1), np.float32)
    mlpb[0, :100, 0] = lbs[0]
    mlpb[1, :100, 0] = lbs[1]
    mlpb[2, :29, 0] = lbs[2]

    shared = dict(waug=waug, mlw1a=mlw1a, mlw1b=mlw1b,
                  mlw2=lws[1].astype(np.float32), mlw3=lws[2].astype(np.float32),
                  iota=iota, ident=ident, biasrep=biasrep, mlpb=mlpb)
    meta = dict(Q1w=tuple(int(v) for v in Q1w), Q2w=tuple(int(v) for v in Q2w),
                N1w=tuple(int(v) for v in n1max), N2w=tuple(int(v) for v in n2max),
                has_bias=bool(has_bias))
    return per_core, shared, meta


def build_nc(Q1w, Q2w, N1w, N2w, has_bias):
    nc = bacc.Bacc("TRN2", target_bir_lowering=False, num_swdge_queues=4)
    SMw = [q1 + q2 for q1, q2 in zip(Q1w, Q2w)]
    SMM = max(SMw)
    NJ = sum(SMw)
    offj_w = np.concatenate([[0], np.cumsum(SMw)]).astype(int)

    xT0 = nc.declare_dram_parameter("xT0", [384, SHARD_PAD], BF16, isOutput=False)
    idxb = nc.declare_dram_parameter("idxb", [P, 8 * NJ], I16, isOutput=False)
    dlocb = nc.declare_dram_parameter("dlocb", [P, NJ], F32, isOutput=False)
    sblob = nc.declare_dram_parameter("sblob", [P, NJ * P], BF16, isOutput=False)
    pmaskb = nc.declare_dram_parameter("pmask", [NT, P, G], BF16, isOutput=False)
    waug = nc.declare_dram_parameter("waug", [11, P, 204], BF16, isOutput=False)
    mlw1a_in = nc.declare_dram_parameter("mlw1a", [P, 100], F32, isOutput=False)
    mlw1b_in = nc.declare_dram_parameter("mlw1b", [P, 100], F32, isOutput=False)
    mlw2_in = nc.declare_dram_parameter("mlw2", [100, 100], F32, isOutput=False)
    mlw3_in = nc.declare_dram_parameter("mlw3", [100, 29], F32, isOutput=False)
    iota_in = nc.declare_dram_parameter("iota", [P, P], BF16, isOutput=False)
    ident_in = nc.declare_dram_parameter("ident", [P, P], BF16, isOutput=False)
    biasrep = nc.declare_dram_parameter("biasrep", [5, P, SCOLS], BF16, isOutput=False)
    mlpb = nc.declare_dram_parameter("mlpb", [3, P, 1], F32, isOutput=False)
    out = nc.declare_dram_parameter("out", [29, G], F32, isOutput=True)

    Tshard = nc.dram_tensor("Tshard", [SHARD_PAD, EL], BF16)
    TfullA = nc.dram_tensor("TfullA", [NFULL_PAD, EL], BF16, addr_space="Shared")
    TfullB = nc.dram_tensor("TfullB", [NFULL_PAD, EL], BF16, addr_space="Shared")
    ed_tab = nc.dram_tensor("ed_tab", [SHARD_PAD, 2], BF16)
    cc2_in = nc.dram_tensor("cc2_in", [SCOLS, G], F32)
    cc2_out = nc.dram_tensor("cc2_out", [SCOLS, G], F32, addr_space="Shared")
    rg = [list(range(NCORE))]

    with tile.TileContext(nc) as tc:
        with tc.tile_pool(name="const", bufs=1) as cpool:
            wtiles = []
            for i in range(11):
                wt = cpool.tile([P, 204], BF16, name=f"waug{i}", tag=f"waug{i}")
                nc.sync.dma_start(out=wt[:], in_=waug[i])
                wtiles.append(wt)
            iota = cpool.tile([P, P], BF16, tag="iota")
            nc.sync.dma_start(out=iota[:], in_=iota_in[:])
            ident = cpool.tile([P, P], BF16, tag="ident")
            nc.sync.dma_start(out=ident[:], in_=ident_in[:])
            if has_bias:
                brts = []
                for li in range(5):
                    brt = cpool.tile([P, SCOLS], BF16, name=f"brt{li}", tag=f"brt{li}")
                    nc.sync.dma_start(out=brt[:], in_=biasrep[li])
                    brts.append(brt)
            # persistent xT ping-pong (SBUF-resident activations)
            xTA = [cpool.tile([P, SHARD_PAD], BF16, name=f"xTA{i}", tag=f"xTA{i}")
                   for i in range(2)]
            xTB = [cpool.tile([SCOLS - P, SHARD_PAD], BF16, name=f"xTB{i}",
                              tag=f"xTB{i}") for i in range(2)]
            # T-row staging ring with baked ones columns
            ts_ring = []
            for i in range(3):
                ts = cpool.tile([P, EL], BF16, name=f"ts{i}", tag=f"ts{i}")
                nc.vector.memset(ts[:], 0.0)
                nc.vector.memset(ts[:, 100:101], 1.0)
                nc.vector.memset(ts[:, 202:203], 1.0)
                ts_ring.append(ts)
            # resident stt for the first WRES windows (reused all 5 layers)
            WRES = 28
            sttres = []
            for w in range(WRES):
                smw = SMw[w]
                ojw = int(offj_w[w])
                srt = cpool.tile([P, smw * P], BF16, name=f"sttres{w}",
                                 tag=f"sttres{w}")
                nc.scalar.dma_start(out=srt[:],
                                    in_=sblob[:, ojw * P:(ojw + smw) * P])
                sttres.append(srt)
            # zero ed_tab pad rows (never written by node phases)
            zpad = cpool.tile([SHARD_PAD - SHARD, 2], BF16, tag="zpad")
            nc.vector.memset(zpad[:], 0.0)
            nc.sync.dma_start(out=ed_tab[SHARD:SHARD_PAD, :], in_=zpad[:])

            with tc.tile_pool(name="sb", bufs=3) as pool, \
                 tc.tile_pool(name="sb3", bufs=3) as pool3:
              with tc.tile_pool(name="ps", bufs=2, space="PSUM") as pspool, \
                   tc.tile_pool(name="psp", bufs=1, space="PSUM") as ppool:

                ppA = ppool.tile([P, G], F32, tag="ppA")
                ppB = ppool.tile([SCOLS - P, G], F32, tag="ppB")

                def node_tile(li, t):
                    r0 = t * P if t < NT - 1 else SHARD - P
                    npsum = ppool.tile([P, 204], F32, tag="npsum")
                    if li == 0:
                        for c in range(3):
                            lt = pool3.tile([P, P], BF16, tag="nlhsT")
                            nc.sync.dma_start(
                                out=lt[:], in_=xT0[c * P:(c + 1) * P, r0:r0 + P])
                            nc.tensor.matmul(
                                out=npsum[:], lhsT=lt[:], rhs=wtiles[c][:],
                                start=(c == 0), stop=(c == 2))
                    else:
                        cur = (li - 1) % 2
                        ca, cb = 3 + 2 * (li - 1), 4 + 2 * (li - 1)
                        nc.tensor.matmul(
                            out=npsum[:], lhsT=xTA[cur][:, r0:r0 + P],
                            rhs=wtiles[ca][:], start=True, stop=False)
                        nc.tensor.matmul(
                            out=npsum[:], lhsT=xTB[cur][:, r0:r0 + P],
                            rhs=wtiles[cb][0:SCOLS - P, :], start=False, stop=True)
                    ts = ts_ring[t % 3]
                    nc.scalar.activation(out=ts[:, 0:100], in_=npsum[:, 0:100],
                                         func=AF.Copy)
                    nc.scalar.activation(out=ts[:, 102:202], in_=npsum[:, 100:200],
                                         func=AF.Copy)
                    nc.vector.tensor_copy(out=ts[:, 204:208].bitcast(F32),
                                          in_=npsum[:, 200:202])
                    edt = pool.tile([P, 2], BF16, tag="edt")
                    nc.vector.tensor_copy(out=edt[:], in_=npsum[:, 202:204])
                    nc.sync.dma_start(out=Tshard[r0:r0 + P, :], in_=ts[:])
                    nc.sync.dma_start(out=ed_tab[r0:r0 + P, :], in_=edt[:])

                qctr = [0]

                state = {}

                def agg_load(li, w, Tf):
                    Q1, Q2 = Q1w[w], Q2w[w]
                    SM = SMw[w]
                    oj = int(offj_w[w])
                    idxt = pool.tile([P, 8 * SMM], I16, tag="idxt")
                    nc.scalar.dma_start(out=idxt[:, 0:8 * SM], in_=idxb[:, 8 * oj:8 * (oj + SM)])
                    if w < WRES:
                        sst = sttres[w]
                    else:
                        sst = pool.tile([P, SMM * P], BF16, tag="sst")
                        nc.scalar.dma_start(out=sst[:, 0:SM * P],
                                            in_=sblob[:, oj * P:(oj + SM) * P])
                    dlt = pool.tile([P, SMM], F32, tag="dlt")
                    nc.scalar.dma_start(out=dlt[:, 0:SM], in_=dlocb[:, oj:oj + SM])
                    edwb = pool.tile([P, 2], BF16, tag="edwb")
                    nc.sync.dma_start(out=edwb[:], in_=ed_tab[w * P:(w + 1) * P, :])
                    Gt = pool.tile([P, SMM, EL], BF16, tag="Gt")
                    if li == 0 and w < 3:
                        nc.vector.memset(Gt[:], 0.0)
                    Q1a = Q1 // 2
                    if Q1a > 0:
                        nc.gpsimd.dma_gather(
                            Gt[:, 0:Q1a, :], Tf[:],
                            idxt[:, 0:8 * Q1a], Q1a * P, Q1a * P, EL,
                            queue_num=qctr[0] % 4); qctr[0] += 1
                    if Q1 - Q1a > 0:
                        nc.gpsimd.dma_gather(
                            Gt[:, Q1a:Q1, :], Tf[:],
                            idxt[:, 8 * Q1a:8 * Q1], (Q1 - Q1a) * P,
                            (Q1 - Q1a) * P, EL,
                            queue_num=qctr[0] % 4); qctr[0] += 1
                    if Q2 > 0:
                        nc.gpsimd.dma_gather(
                            Gt[:, Q1:SM, :], Tf[SPLIT:NFULL_PAD, :],
                            idxt[:, 8 * Q1:8 * SM], Q2 * P, Q2 * P, EL,
                            queue_num=qctr[0] % 4); qctr[0] += 1
                    state[w] = dict(sst=sst, dlt=dlt, edwb=edwb, Gt=Gt)

                def agg_score(li, w):
                    SM = SMw[w]
                    s = state[w]
                    sst, dlt, edwb, Gt = s["sst"], s["dlt"], s["edwb"], s["Gt"]
                    st = pool.tile([P, SMM * P], BF16, tag="st")
                    for j in range(SM):
                        nc.vector.tensor_scalar(
                            out=st[:, j * P:(j + 1) * P], in0=iota[:],
                            scalar1=dlt[:, j:j + 1], scalar2=None,
                            op0=ALU.is_equal)
                    s["st"] = st
                    edp = pspool.tile([P, 2 * SMM], F32, tag="edp")
                    for j in range(SM):
                        nc.tensor.matmul(
                            out=edp[:, 2 * j:2 * j + 2],
                            lhsT=sst[:, j * P:(j + 1) * P], rhs=edwb[:],
                            start=True, stop=True)
                    ev = pool.tile([P, SMM, 2], F32, tag="ev")
                    nc.vector.tensor_add(
                        out=ev[:, 0:SM, :], in0=Gt[:, 0:SM, 204:208].bitcast(F32),
                        in1=edp[:, 0:2 * SM].rearrange("p (j c) -> p j c", c=2))
                    wv = pool.tile([P, SMM, 2], F32, tag="wv")
                    nc.scalar.activation(out=wv[:, 0:SM, :], in_=ev[:, 0:SM, :],
                                         func=AF.Prelu, alpha=SLOPE)
                    nc.scalar.activation(out=wv[:, 0:SM, :], in_=wv[:, 0:SM, :],
                                         func=AF.Exp)
                    # in-place Vt = [w*h | w]
                    nc.vector.tensor_mul(
                        out=Gt[:, 0:SM, 0:101], in0=Gt[:, 0:SM, 0:101],
                        in1=wv[:, 0:SM, 0:1].broadcast_to([P, SM, 101]))
                    nc.vector.tensor_mul(
                        out=Gt[:, 0:SM, 102:203], in0=Gt[:, 0:SM, 102:203],
                        in1=wv[:, 0:SM, 1:2].broadcast_to([P, SM, 101]))

                def agg_reduce(li, w):
                    last = li == 4
                    SM = SMw[w]
                    s = state.pop(w)
                    st, Gt = s["st"], s["Gt"]
                    apsum = pspool.tile([P, SCOLS], F32, tag="apsum")
                    for j in range(SM):
                        nc.tensor.matmul(
                            out=apsum[:], lhsT=st[:, j * P:(j + 1) * P],
                            rhs=Gt[:, j, 0:SCOLS],
                            start=(j == 0), stop=(j == SM - 1))
                    sc = pool.tile([P, 2], F32, tag="sc")
                    nc.vector.tensor_scalar_add(
                        out=sc[:], in0=apsum[:, 100:SCOLS:102], scalar1=1e-30)
                    rc = pool.tile([P, 2], F32, tag="rc")
                    nc.vector.reciprocal(out=rc[:], in_=sc[:])
                    stb = pool.tile([P, SCOLS], BF16, tag="stb")
                    if has_bias:
                        sg = pool.tile([P, SCOLS], F32, tag="sg")
                        nc.vector.tensor_scalar_mul(
                            out=sg[:, 0:101], in0=apsum[:, 0:101], scalar1=rc[:, 0:1])
                        nc.vector.tensor_scalar_mul(
                            out=sg[:, 101:SCOLS], in0=apsum[:, 101:SCOLS],
                            scalar1=rc[:, 1:2])
                        nc.vector.tensor_add(out=sg[:], in0=sg[:], in1=brts[li][:])
                        nc.scalar.activation(out=stb[:], in_=sg[:], func=AF.Relu)
                    else:
                        nc.scalar.activation(out=stb[:, 0:101], in_=apsum[:, 0:101],
                                             func=AF.Relu, scale=rc[:, 0:1])
                        nc.scalar.activation(out=stb[:, 101:SCOLS],
                                             in_=apsum[:, 101:SCOLS],
                                             func=AF.Relu, scale=rc[:, 1:2])
                    if not last:
                        nxt = li % 2
                        tp = ppool.tile([P, 2 * P], BF16, tag="tp")
                        nc.tensor.transpose(out=tp[:, 0:P], in_=stb[:, 0:P],
                                            identity=ident[:])
                        nc.tensor.transpose(out=tp[0:SCOLS - P, P:2 * P],
                                            in_=stb[:, P:SCOLS], identity=ident[:])
                        c0 = w * P
                        nc.scalar.activation(out=xTA[nxt][:, c0:c0 + P],
                                             in_=tp[:, 0:P], func=AF.Copy)
                        nc.scalar.activation(out=xTB[nxt][:, c0:c0 + P],
                                             in_=tp[0:SCOLS - P, P:2 * P],
                                             func=AF.Copy)
                    else:
                        pmt = pool.tile([P, G], BF16, tag="pmt")
                        nc.scalar.dma_start(out=pmt[:], in_=pmaskb[w])
                        nc.tensor.matmul(out=ppA[:], lhsT=stb[:, 0:P], rhs=pmt[:],
                                         start=(w == 0), stop=(w == NW - 1))
                        nc.tensor.matmul(out=ppB[:], lhsT=stb[:, P:SCOLS],
                                         rhs=pmt[:],
                                         start=(w == 0), stop=(w == NW - 1))

                # ---- schedule: node0; per layer chunked CC + 3-stage pipeline ----
                for t in range(NT):
                    node_tile(0, t)
                for li in range(5):
                    Tf = TfullA if li % 2 == 0 else TfullB
                    nc.gpsimd.collective_compute(
                        "AllGather", ALU.bypass, replica_groups=rg,
                        ins=[Tshard[0:RHALF, :]], outs=[Tf[0:CHA, :]])
                    nc.gpsimd.collective_compute(
                        "AllGather", ALU.bypass, replica_groups=rg,
                        ins=[Tshard[RHALF:SHARD_PAD, :]],
                        outs=[Tf[CHA:NFULL_PAD, :]])
                    for w in range(NW + 2):
                        if w < NW:
                            agg_load(li, w, Tf)
                        if 1 <= w <= NW:
                            agg_score(li, w - 1)
                        if w >= 2:
                            agg_reduce(li, w - 2)
                            if li < 4:
                                node_tile(li + 1, w - 2)

                # ---- pool tail: AllReduce + MLP ----
                cpA = pool.tile([P, G], F32, tag="cpA")
                nc.vector.tensor_copy(out=cpA[:], in_=ppA[:])
                cpB = pool.tile([SCOLS - P, G], F32, tag="cpB")
                nc.vector.tensor_copy(out=cpB[:], in_=ppB[:])
                nc.sync.dma_start(out=cc2_in[0:P, :], in_=cpA[:])
                nc.sync.dma_start(out=cc2_in[P:SCOLS, :], in_=cpB[:])
              with tc.tile_pool(name="ps2", bufs=1, space="PSUM") as pspool:
                tc.strict_bb_all_engine_barrier()
                nc.gpsimd.collective_compute(
                    "AllReduce", ALU.add, replica_groups=rg,
                    ins=[cc2_in[:]], outs=[cc2_out[:]])
                tc.strict_bb_all_engine_barrier()
                plA = pool.tile([P, G], F32, tag="plA")
                nc.sync.dma_start(out=plA[:], in_=cc2_out[0:P, :])
                plB = pool.tile([SCOLS - P, G], F32, tag="plB")
                nc.sync.dma_start(out=plB[:], in_=cc2_out[P:SCOLS, :])
                w1a = pool.tile([P, 100], F32, tag="w1a")
                nc.sync.dma_start(out=w1a[:], in_=mlw1a_in[:])
                w1b = pool.tile([SCOLS - P, 100], F32, tag="w1b")
                nc.sync.dma_start(out=w1b[:], in_=mlw1b_in[0:SCOLS - P, :])
                w2t = pool.tile([100, 100], F32, tag="w2t")
                nc.sync.dma_start(out=w2t[:], in_=mlw2_in[:])
                w3t = pool.tile([100, 29], F32, tag="w3t")
                nc.sync.dma_start(out=w3t[:], in_=mlw3_in[:])
                if has_bias:
                    b1 = pool.tile([P, 1], F32, tag="b1")
                    nc.sync.dma_start(out=b1[:], in_=mlpb[0])
                    b2 = pool.tile([P, 1], F32, tag="b2")
                    nc.sync.dma_start(out=b2[:], in_=mlpb[1])
                    b3 = pool.tile([P, 1], F32, tag="b3")
                    nc.sync.dma_start(out=b3[:], in_=mlpb[2])
                y1p = pspool.tile([100, G], F32, tag="y1p")
                nc.tensor.matmul(out=y1p[:], lhsT=w1a[:], rhs=plA[:],
                                 start=True, stop=False)
                nc.tensor.matmul(out=y1p[:], lhsT=w1b[:], rhs=plB[:],
                                 start=False, stop=True)
                y1 = pool.tile([100, G], F32, tag="y1")
                nc.scalar.activation(out=y1[:], in_=y1p[:], func=AF.Relu,
                                     bias=b1[0:100, :] if has_bias else 0.0)
                y2p = pspool.tile([100, G], F32, tag="y2p")
                nc.tensor.matmul(out=y2p[:], lhsT=w2t[:], rhs=y1[:],
                                 start=True, stop=True)
                y2 = pool.tile([100, G], F32, tag="y2")
                nc.scalar.activation(out=y2[:], in_=y2p[:], func=AF.Relu,
                                     bias=b2[0:100, :] if has_bias else 0.0)
                y3p = pspool.tile([29, G], F32, tag="y3p")
                nc.tensor.matmul(out=y3p[:], lhsT=w3t[:], rhs=y2[:],
                                 start=True, stop=True)
                y3 = pool.tile([29, G], F32, tag="y3")
                nc.scalar.activation(out=y3[:], in_=y3p[:], func=AF.Identity,
                                     bias=b3[0:29, :] if has_bias else 0.0)
                nc.sync.dma_start(out=out[:], in_=y3[:])

    nc.finalize()
    return nc


def make_in_maps(per_core, shared):
    return [{**pc, **shared} for pc in per_core]


# ---------------- runner (device-resident SPMD invoke) ----------------
import jax
from jax.sharding import Mesh, PartitionSpec, NamedSharding
from jax.experimental.shard_map import shard_map
from concourse import bass2jax
from concourse.bass2jax import _bass_exec_p, install_neuronx_cc_hook, partition_id_tensor


class SpmdRunner:
    def __init__(self, nc, n_cores=8):
        install_neuronx_cc_hook()
        self.nc = nc
        self.n_cores = n_cores
        partition_name = nc.partition_id_tensor.name if nc.partition_id_tensor else None
        in_names, out_names, out_avals, zero_outs = [], [], [], []
        for alloc in nc.m.functions[0].allocations:
            if not isinstance(alloc, mybir.MemoryLocationSet):
                continue
            name = alloc.memorylocations[0].name
            if alloc.kind == "ExternalInput":
                if name != partition_name and name != (nc.dbg_addr.name if nc.dbg_addr else None):
                    in_names.append(name)
            elif alloc.kind == "ExternalOutput":
                out_names.append(name)
                shape = tuple(alloc.tensor_shape)
                dtype = mybir.dt.np(alloc.dtype)
                out_avals.append(jax.core.ShapedArray(shape, dtype))
                zero_outs.append(np.zeros(shape, dtype))
        self.in_names, self.out_names = in_names, out_names
        self.out_avals, self.zero_outs = out_avals, zero_outs
        n_params, n_outs = len(in_names), len(out_names)
        self.n_params = n_params
        all_in_names = list(in_names) + list(out_names)
        if nc.dbg_addr is not None:
            all_in_names.append(nc.dbg_addr.name)
        if partition_name is not None:
            all_in_names.append(partition_name)
        self.has_dbg = nc.dbg_addr is not None

        def _body(*args):
            operands = list(args)
            if self.has_dbg:
                operands.append(jax.numpy.zeros((1, 2), jax.numpy.uint32))
            if partition_name is not None:
                operands.append(partition_id_tensor())
            outs = _bass_exec_p.bind(
                *operands,
                out_avals=tuple(out_avals),
                in_names=tuple(all_in_names),
                out_names=tuple(out_names),
                lowering_input_output_aliases=(),
                sim_require_finite=False,
                sim_require_nnan=False,
                nc=nc,
            )
            return tuple(outs)

        devices = jax.devices()[:n_cores]
        self.mesh = Mesh(np.asarray(devices), ("core",))
        in_specs = (PartitionSpec("core"),) * (n_params + n_outs)
        out_specs = (PartitionSpec("core"),) * n_outs
        donate = tuple(range(n_params, n_params + n_outs))
        self.sharded = jax.jit(
            shard_map(_body, mesh=self.mesh, in_specs=in_specs,
                      out_specs=out_specs, check_rep=False),
            donate_argnums=donate, keep_unused=True,
        )
        self.sharding = NamedSharding(self.mesh, PartitionSpec("core"))
        self.dev_in = None

    def stage_inputs(self, in_maps):
        per_core = [[np.asarray(m[n]) for n in self.in_names] for m in in_maps]
        concat_in = [
            np.concatenate([per_core[c][i] for c in range(self.n_cores)], axis=0)
            for i in range(self.n_params)
        ]
        self.dev_in = [jax.device_put(a, self.sharding) for a in concat_in]
        for a in self.dev_in:
            a.block_until_ready()

    def __call__(self):
        concat_zeros = [
            jax.device_put(
                np.zeros((self.n_cores * z.shape[0], *z.shape[1:]), z.dtype),
                self.sharding)
            for z in self.zero_outs
        ]
        out = self.sharded(*self.dev_in, *concat_zeros)
        for o in out:
            o.block_until_ready()
        return out

    def results(self, out):
        return [
            {
                name: np.asarray(out[i]).reshape(self.n_cores, *self.out_avals[i].shape)[c]
                for i, name in enumerate(self.out_names)
            }
            for c in range(self.n_cores)
        ]


# ---------------- entry point ----------------
_CACHE = {}


def _get_runner(meta):
    key = (meta["Q1w"], meta["Q2w"], meta["N1w"], meta["N2w"], meta["has_bias"])
    if key not in _CACHE:
        nc = build_nc(list(meta["Q1w"]), list(meta["Q2w"]),
                      list(meta["N1w"]), list(meta["N2w"]), meta["has_bias"])
        _CACHE[key] = SpmdRunner(nc, NCORE)
    return _CACHE[key]


def kernel(**inputs):
    x = np.asarray(inputs["x"], np.float32)
    edge_index = np.asarray(inputs["edge_index"])
    batch = np.asarray(inputs["batch"])
    Ws = [np.asarray(inputs[f"W{i+1}"], np.float32) for i in range(5)]
    asrcs = [np.asarray(inputs[f"asrc{i+1}"], np.float32) for i in range(5)]
    adsts = [np.asarray(inputs[f"adst{i+1}"], np.float32) for i in range(5)]
    bcs = [np.asarray(inputs[f"bc{i+1}"], np.float32) for i in range(5)]
    lws = [np.asarray(inputs[f"lw{i+1}"], np.float32) for i in range(3)]
    lbs = [np.asarray(inputs[f"lb{i+1}"], np.float32) for i in range(3)]
    per_core, shared, meta = prep(x, edge_index, batch, Ws, asrcs, adsts, bcs, lws, lbs)
    r = _get_runner(meta)
    r.stage_inputs(make_in_maps(per_core, shared))
    out = r()
    y3 = r.results(out)[0]["out"]       # [29, 256]
    return np.ascontiguousarray(y3.T)   # [256, 29]


# revision 17
# speedup vs baseline: 1.4821x; 1.4821x over previous
"""GATNet Trainium kernel v2: host preprocessing + Bass program builder.

Design (8-way dst-shard of nodes, bf16 throughout):
  node phase (per-shard): npsum = xT_chunk.T @ [Wh0|Wh1|was|wad] (PE bf16)
    -> staged into 256-col T rows [h0|1|0|h1|1|0|es(f32 @204:208)|pad] + ed_tab (bf16)
  exchange: AllGather of Tshard [6272, 256] bf16 -> Tfull (ping-pong A/B)
  aggregation (per-shard dst windows of 128 dsts):
    - ONE dma_gather per window half (int16 idx limit splits rows at 32768)
      pulls all ~11*128 edge rows (512B each) in 2 SWDGE instructions
    - edp = stt_j^T @ ed (PE, one-hot from DRAM stblob bf16)
    - e = es_src + ed_dst (DVE), w = Exp(Prelu(e)) (ACT)
    - in-place Vt: Gt[h|1] *= w per head (DVE broadcast mul)
    - segment-sum: apsum += st_j^T @ Gt_j[0:203] (PE, one j-matmul per tile)
    - epilogue: ACT Relu(apsum * 1/s) per head -> stb bf16; PE transpose ->
      persistent SBUF xT buffers (no DRAM roundtrip)
  pool phase fused into last agg layer: ppA/ppB += stb^T @ pmask(rcnt-baked),
  AllReduce [203, 256] f32, 3-layer MLP on PE/ACT.
"""
import sys
sys.path.insert(0, "/opt/trn_rl_repo")
sys.path.insert(0, "/opt/trn_rl_repo/concourse")
import numpy as np
import ml_dtypes
from concourse import bass, bacc, mybir
import concourse.tile as tile

BF16 = mybir.dt.bfloat16
F32 = mybir.dt.float32
I16 = mybir.dt.int16
AF = mybir.ActivationFunctionType
ALU = mybir.AluOpType
BF = ml_dtypes.bfloat16

N = 50000
E = 400000
G = 256
H = 2
C = 100
F_IN = 336
SLOPE = 0.2
NCORE = 8
SHARD = N // NCORE            # 6250
P = 128
SHARD_PAD = 6272              # 49*128
NW = 49
NT = 49
NFULL_PAD = NCORE * SHARD_PAD # 50176
RHALF = 3200                  # rows of chunk A (node tiles 0:24)
CHA = NCORE * RHALF           # 25600 rows of chunk-A block in Tfull
SPLIT = 32768                 # int16 idx limit for dma_gather
EL = 256                      # T row cols (bf16) = 512B
SCOLS = 203                   # stb cols: [x0(100)|s0|z|x1(100)|s1]


def _wrap16(vals):
    """int16 idx list -> [128, ceil(n/16)] wrapped layout (idx i at [i%16, i//16])."""
    n = len(vals)
    k = -(-n // 16)
    arr = np.zeros((16, k), np.int16)
    arr[np.arange(n) % 16, np.arange(n) // 16] = vals
    return np.tile(arr, (8, 1))


def prep(x, edge_index, batch, Ws, asrcs, adsts, bcs, lws, lbs):
    src_all = np.concatenate([edge_index[0], np.arange(N, dtype=np.int64)])
    dst_all = np.concatenate([edge_index[1], np.arange(N, dtype=np.int64)])
    order = np.argsort(dst_all, kind="stable")
    s_sorted = src_all[order].astype(np.int64)
    d_sorted = dst_all[order].astype(np.int64)
    s_k = s_sorted // SHARD
    s_r = s_sorted % SHARD
    s_row = np.where(s_r < RHALF, s_k * RHALF + s_r,
                     CHA + s_k * (SHARD_PAD - RHALF) + (s_r - RHALF))

    # per (core, window) edge ranges + lo/hi split sizes
    win = {}
    q1_req = np.zeros((NCORE, NW), np.int64)
    q2_req = np.zeros((NCORE, NW), np.int64)
    for k in range(NCORE):
        for w in range(NW):
            d_lo = k * SHARD + w * P
            d_hi = min(k * SHARD + (w + 1) * P, (k + 1) * SHARD)
            lo = np.searchsorted(d_sorted, d_lo, side="left")
            hi = np.searchsorted(d_sorted, d_hi, side="left")
            sr = s_row[lo:hi]
            m = sr < SPLIT
            win[(k, w)] = (lo, hi, m)
            q1_req[k, w] = int(m.sum())
            q2_req[k, w] = int((~m).sum())
    n1max = q1_req.max(axis=0)
    n2max = q2_req.max(axis=0)
    Q1w = -(-n1max // P)
    Q2w = -(-n2max // P)
    SMw = Q1w + Q2w
    NJ = int(SMw.sum())

    per_core = []
    for k in range(NCORE):
        idxb = np.zeros((P, 8 * NJ), np.int16)
        dlocb = np.full((P, NJ), -1.0, np.float32)
        sb = np.zeros((P, NJ * P), BF)
        offj = 0
        for w in range(NW):
            lo, hi, m = win[(k, w)]
            Q1, Q2 = int(Q1w[w]), int(Q2w[w])
            SM = Q1 + Q2
            sr = s_row[lo:hi]
            dl = d_sorted[lo:hi] - (k * SHARD + w * P)
            n1 = int(m.sum())
            n2 = int((~m).sum())
            olo = np.argsort(sr[m], kind="stable")
            ohi = np.argsort(sr[~m], kind="stable")
            ilo = np.zeros(Q1 * P, np.int64)
            ilo[:n1] = sr[m][olo]
            ihi = np.zeros(Q2 * P, np.int64)
            ihi[:n2] = sr[~m][ohi] - SPLIT
            wrapped = np.concatenate(
                [_wrap16(ilo.astype(np.int16)), _wrap16(ihi.astype(np.int16))]
                if Q2 > 0 else [_wrap16(ilo.astype(np.int16))], axis=1)
            idxb[:, 8 * offj:8 * (offj + SM)] = wrapped
            dslot = np.full(SM * P, -1, np.int64)
            dslot[:n1] = dl[m][olo]
            dslot[Q1 * P:Q1 * P + n2] = dl[~m][ohi]
            dlocb[:, offj:offj + SM] = dslot.reshape(SM, P).T.astype(np.float32)
            # transposed one-hot stt_j[mm, p] for the ed matmul
            oh = np.zeros((SM * P, P), np.float32)
            valid = dslot >= 0
            oh[np.arange(SM * P)[valid], dslot[valid]] = 1.0
            oh3 = oh.reshape(SM, P, P)
            sb[:, offj * P:(offj + SM) * P] = (
                oh3.transpose(2, 0, 1).reshape(P, SM * P).astype(BF))
            offj += SM
        xT0 = np.zeros((384, SHARD_PAD), BF)
        xT0[:F_IN, :SHARD] = x[k * SHARD:(k + 1) * SHARD].T.astype(BF)
        # pool mask with 1/cnt baked in
        cnt = np.bincount(batch, minlength=G).astype(np.float32)
        rcnt = 1.0 / np.maximum(cnt, 1.0)
        pmask = np.zeros((NT, P, G), BF)
        bsh = batch[k * SHARD:(k + 1) * SHARD]
        for t in range(NT):
            r0, r1 = t * P, min(t * P + P, SHARD)
            if r1 > r0:
                pmask[t, np.arange(r1 - r0), bsh[r0:r1]] = rcnt[bsh[r0:r1]].astype(BF)
        per_core.append(dict(xT0=xT0, idxb=idxb, dlocb=dlocb, sblob=sb,
                             pmask=pmask))

    # weights: npsum cols [h0(100)|h1(100)|es0 es1|ed0 ed1] = 204
    def make_aug(W, a_s, a_d):
        F = W.shape[0]
        was = np.zeros((F, 2), np.float32)
        wad = np.zeros((F, 2), np.float32)
        for h in range(H):
            was[:, h] = W[:, h * C:(h + 1) * C] @ a_s[h]
            wad[:, h] = W[:, h * C:(h + 1) * C] @ a_d[h]
        return np.concatenate(
            [W[:, 0:C], W[:, C:2 * C], was, wad], axis=1)  # [F, 204]

    # L0: 3 chunks of rows (336 padded to 384)
    waug = np.zeros((11, P, 204), BF)
    aug0 = make_aug(Ws[0].astype(np.float64), asrcs[0], adsts[0]).astype(np.float32)
    for c in range(3):
        rows = aug0[c * P:min((c + 1) * P, F_IN)]
        waug[c, :rows.shape[0]] = rows.astype(BF)
    # L1-4: features live at stb cols (f<100 -> col f; f>=100 -> col f+2)
    for li in range(1, 5):
        aug = make_aug(Ws[li].astype(np.float64), asrcs[li], adsts[li]).astype(np.float32)
        perm = np.zeros((SCOLS, 204), np.float32)
        perm[0:100] = aug[0:100]
        perm[102:202] = aug[100:200]
        waug[3 + 2 * (li - 1), :, :] = perm[0:128].astype(BF)
        waug[4 + 2 * (li - 1), 0:SCOLS - 128, :] = perm[128:SCOLS].astype(BF)

    # MLP weights with same row-perm for lw1
    w1p = np.zeros((SCOLS, 100), np.float32)
    w1p[0:100] = lws[0][0:100]
    w1p[102:202] = lws[0][100:200]
    mlw1a = w1p[0:128].astype(np.float32)
    mlw1b = np.zeros((P, 100), np.float32)
    mlw1b[0:SCOLS - 128] = w1p[128:SCOLS]
    iota = np.broadcast_to(np.arange(P, dtype=np.float32), (P, P)).astype(BF)
    ident = np.eye(P, dtype=BF)

    has_bias = any(np.abs(b).max() > 0 for b in bcs) or any(
        np.abs(b).max() > 0 for b in lbs)
    biasrep = np.zeros((5, P, SCOLS), BF)
    for li in range(5):
        biasrep[li, :, 0:100] = bcs[li][0:100]
        biasrep[li, :, 102:202] = bcs[li][100:200]
    mlpb = np.zeros((3, P, 1), np.float32)
    mlpb[0, :100, 0] = lbs[0]
    mlpb[1, :100, 0] = lbs[1]
    mlpb[2, :29, 0] = lbs[2]

    shared = dict(waug=waug, mlw1a=mlw1a, mlw1b=mlw1b,
                  mlw2=lws[1].astype(np.float32), mlw3=lws[2].astype(np.float32),
                  iota=iota, ident=ident, biasrep=biasrep, mlpb=mlpb)
    meta = dict(Q1w=tuple(int(v) for v in Q1w), Q2w=tuple(int(v) for v in Q2w),
                N1w=tuple(int(v) for v in n1max), N2w=tuple(int(v) for v in n2max),
                has_bias=bool(has_bias))
    return per_core, shared, meta


def build_nc(Q1w, Q2w, N1w, N2w, has_bias):
    nc = bacc.Bacc("TRN2", target_bir_lowering=False, num_swdge_queues=4)
    SMw = [q1 + q2 for q1, q2 in zip(Q1w, Q2w)]
    SMM = max(SMw)
    NJ = sum(SMw)
    offj_w = np.concatenate([[0], np.cumsum(SMw)]).astype(int)

    xT0 = nc.declare_dram_parameter("xT0", [384, SHARD_PAD], BF16, isOutput=False)
    idxb = nc.declare_dram_parameter("idxb", [P, 8 * NJ], I16, isOutput=False)
    dlocb = nc.declare_dram_parameter("dlocb", [P, NJ], F32, isOutput=False)
    sblob = nc.declare_dram_parameter("sblob", [P, NJ * P], BF16, isOutput=False)
    pmaskb = nc.declare_dram_parameter("pmask", [NT, P, G], BF16, isOutput=False)
    waug = nc.declare_dram_parameter("waug", [11, P, 204], BF16, isOutput=False)
    mlw1a_in = nc.declare_dram_parameter("mlw1a", [P, 100], F32, isOutput=False)
    mlw1b_in = nc.declare_dram_parameter("mlw1b", [P, 100], F32, isOutput=False)
    mlw2_in = nc.declare_dram_parameter("mlw2", [100, 100], F32, isOutput=False)
    mlw3_in = nc.declare_dram_parameter("mlw3", [100, 29], F32, isOutput=False)
    iota_in = nc.declare_dram_parameter("iota", [P, P], BF16, isOutput=False)
    ident_in = nc.declare_dram_parameter("ident", [P, P], BF16, isOutput=False)
    biasrep = nc.declare_dram_parameter("biasrep", [5, P, SCOLS], BF16, isOutput=False)
    mlpb = nc.declare_dram_parameter("mlpb", [3, P, 1], F32, isOutput=False)
    out = nc.declare_dram_parameter("out", [29, G], F32, isOutput=True)

    Tshard = nc.dram_tensor("Tshard", [SHARD_PAD, EL], BF16)
    TfullA = nc.dram_tensor("TfullA", [NFULL_PAD, EL], BF16, addr_space="Shared")
    TfullB = nc.dram_tensor("TfullB", [NFULL_PAD, EL], BF16, addr_space="Shared")
    ed_tab = nc.dram_tensor("ed_tab", [SHARD_PAD, 2], BF16)
    cc2_in = nc.dram_tensor("cc2_in", [SCOLS, G], F32)
    cc2_out = nc.dram_tensor("cc2_out", [SCOLS, G], F32, addr_space="Shared")
    rg = [list(range(NCORE))]

    with tile.TileContext(nc) as tc:
        with tc.tile_pool(name="const", bufs=1) as cpool:
            wtiles = []
            for i in range(11):
                wt = cpool.tile([P, 204], BF16, name=f"waug{i}", tag=f"waug{i}")
                nc.sync.dma_start(out=wt[:], in_=waug[i])
                wtiles.append(wt)
            iota = cpool.tile([P, P], BF16, tag="iota")
            nc.sync.dma_start(out=iota[:], in_=iota_in[:])
            ident = cpool.tile([P, P], BF16, tag="ident")
            nc.sync.dma_start(out=ident[:], in_=ident_in[:])
            if has_bias:
                brts = []
                for li in range(5):
                    brt = cpool.tile([P, SCOLS], BF16, name=f"brt{li}", tag=f"brt{li}")
                    nc.sync.dma_start(out=brt[:], in_=biasrep[li])
                    brts.append(brt)
            # persistent xT ping-pong (SBUF-resident activations)
            xTA = [cpool.tile([P, SHARD_PAD], BF16, name=f"xTA{i}", tag=f"xTA{i}")
                   for i in range(2)]
            xTB = [cpool.tile([SCOLS - P, SHARD_PAD], BF16, name=f"xTB{i}",
                              tag=f"xTB{i}") for i in range(2)]
            # T-row staging ring with baked ones columns
            ts_ring = []
            for i in range(3):
                ts = cpool.tile([P, EL], BF16, name=f"ts{i}", tag=f"ts{i}")
                nc.vector.memset(ts[:], 0.0)
                nc.vector.memset(ts[:, 100:101], 1.0)
                nc.vector.memset(ts[:, 202:203], 1.0)
                ts_ring.append(ts)
            # resident stt for the first WRES windows (reused all 5 layers)
            WRES = 28
            sttres = []
            for w in range(WRES):
                smw = SMw[w]
                ojw = int(offj_w[w])
                srt = cpool.tile([P, smw * P], BF16, name=f"sttres{w}",
                                 tag=f"sttres{w}")
                nc.scalar.dma_start(out=srt[:],
                                    in_=sblob[:, ojw * P:(ojw + smw) * P])
                sttres.append(srt)
            # zero ed_tab pad rows (never written by node phases)
            zpad = cpool.tile([SHARD_PAD - SHARD, 2], BF16, tag="zpad")
            nc.vector.memset(zpad[:], 0.0)
            nc.sync.dma_start(out=ed_tab[SHARD:SHARD_PAD, :], in_=zpad[:])

            with tc.tile_pool(name="sb", bufs=3) as pool, \
                 tc.tile_pool(name="sb3", bufs=3) as pool3:
              with tc.tile_pool(name="ps", bufs=2, space="PSUM") as pspool, \
                   tc.tile_pool(name="psp", bufs=1, space="PSUM") as ppool:

                ppA = ppool.tile([P, G], F32, tag="ppA")
                ppB = ppool.tile([SCOLS - P, G], F32, tag="ppB")

                def node_tile(li, t):
                    r0 = t * P if t < NT - 1 else SHARD - P
                    npsum = ppool.tile([P, 204], F32, tag="npsum")
                    if li == 0:
                        for c in range(3):
                            lt = pool3.tile([P, P], BF16, tag="nlhsT")
                            nc.sync.dma_start(
                                out=lt[:], in_=xT0[c * P:(c + 1) * P, r0:r0 + P])
                            nc.tensor.matmul(
                                out=npsum[:], lhsT=lt[:], rhs=wtiles[c][:],
                                start=(c == 0), stop=(c == 2))
                    else:
                        cur = (li - 1) % 2
                        ca, cb = 3 + 2 * (li - 1), 4 + 2 * (li - 1)
                        nc.tensor.matmul(
                            out=npsum[:], lhsT=xTA[cur][:, r0:r0 + P],
                            rhs=wtiles[ca][:], start=True, stop=False)
                        nc.tensor.matmul(
                            out=npsum[:], lhsT=xTB[cur][:, r0:r0 + P],
                            rhs=wtiles[cb][0:SCOLS - P, :], start=False, stop=True)
                    ts = ts_ring[t % 3]
                    nc.scalar.activation(out=ts[:, 0:100], in_=npsum[:, 0:100],
                                         func=AF.Copy)
                    nc.scalar.activation(out=ts[:, 102:202], in_=npsum[:, 100:200],
                                         func=AF.Copy)
                    nc.vector.tensor_copy(out=ts[:, 204:208].bitcast(F32),
                                          in_=npsum[:, 200:202])
                    edt = pool.tile([P, 2], BF16, tag="edt")
                    nc.vector.tensor_copy(out=edt[:], in_=npsum[:, 202:204])
                    nc.sync.dma_start(out=Tshard[r0:r0 + P, :], in_=ts[:])
                    nc.sync.dma_start(out=ed_tab[r0:r0 + P, :], in_=edt[:])

                qctr = [0]

                state = {}

                def agg_load(li, w, Tf):
                    Q1, Q2 = Q1w[w], Q2w[w]
                    SM = SMw[w]
                    oj = int(offj_w[w])
                    idxt = pool.tile([P, 8 * SMM], I16, tag="idxt")
                    nc.scalar.dma_start(out=idxt[:, 0:8 * SM], in_=idxb[:, 8 * oj:8 * (oj + SM)])
                    if w < WRES:
                        sst = sttres[w]
                    else:
                        sst = pool.tile([P, SMM * P], BF16, tag="sst")
                        nc.scalar.dma_start(out=sst[:, 0:SM * P],
                                            in_=sblob[:, oj * P:(oj + SM) * P])
                    dlt = pool.tile([P, SMM], F32, tag="dlt")
                    nc.scalar.dma_start(out=dlt[:, 0:SM], in_=dlocb[:, oj:oj + SM])
                    edwb = pool.tile([P, 2], BF16, tag="edwb")
                    nc.sync.dma_start(out=edwb[:], in_=ed_tab[w * P:(w + 1) * P, :])
                    Gt = pool.tile([P, SMM, EL], BF16, tag="Gt")
                    if li == 0 and w < 3:
                        nc.vector.memset(Gt[:], 0.0)
                    Q1a = Q1 // 2
                    if Q1a > 0:
                        nc.gpsimd.dma_gather(
                            Gt[:, 0:Q1a, :], Tf[:],
                            idxt[:, 0:8 * Q1a], Q1a * P, Q1a * P, EL,
                            queue_num=qctr[0] % 4); qctr[0] += 1
                    if Q1 - Q1a > 0:
                        nc.gpsimd.dma_gather(
                            Gt[:, Q1a:Q1, :], Tf[:],
                            idxt[:, 8 * Q1a:8 * Q1], (Q1 - Q1a) * P,
                            (Q1 - Q1a) * P, EL,
                            queue_num=qctr[0] % 4); qctr[0] += 1
                    if Q2 > 0:
                        nc.gpsimd.dma_gather(
                            Gt[:, Q1:SM, :], Tf[SPLIT:NFULL_PAD, :],
                            idxt[:, 8 * Q1:8 * SM], Q2 * P, Q2 * P, EL,
                            queue_num=qctr[0] % 4); qctr[0] += 1
                    st = pool.tile([P, SMM * P], BF16, tag="st")
                    for j in range(SM):
                        nc.vector.tensor_scalar(
                            out=st[:, j * P:(j + 1) * P], in0=iota[:],
                            scalar1=dlt[:, j:j + 1], scalar2=None,
                            op0=ALU.is_equal)
                    state[w] = dict(sst=sst, st=st, edwb=edwb, Gt=Gt)

                def agg_score(li, w):
                    SM = SMw[w]
                    s = state[w]
                    sst, edwb, Gt = s["sst"], s["edwb"], s["Gt"]
                    edp = pspool.tile([P, 2 * SMM], F32, tag="edp")
                    for j in range(SM):
                        nc.tensor.matmul(
                            out=edp[:, 2 * j:2 * j + 2],
                            lhsT=sst[:, j * P:(j + 1) * P], rhs=edwb[:],
                            start=True, stop=True)
                    ev = pool.tile([P, SMM, 2], F32, tag="ev")
                    nc.vector.tensor_add(
                        out=ev[:, 0:SM, :], in0=Gt[:, 0:SM, 204:208].bitcast(F32),
                        in1=edp[:, 0:2 * SM].rearrange("p (j c) -> p j c", c=2))
                    wv = pool.tile([P, SMM, 2], F32, tag="wv")
                    nc.scalar.activation(out=wv[:, 0:SM, :], in_=ev[:, 0:SM, :],
                                         func=AF.Prelu, alpha=SLOPE)
                    nc.scalar.activation(out=wv[:, 0:SM, :], in_=wv[:, 0:SM, :],
                                         func=AF.Exp)
                    # in-place Vt = [w*h | w]
                    nc.vector.tensor_mul(
                        out=Gt[:, 0:SM, 0:101], in0=Gt[:, 0:SM, 0:101],
                        in1=wv[:, 0:SM, 0:1].broadcast_to([P, SM, 101]))
                    nc.vector.tensor_mul(
                        out=Gt[:, 0:SM, 102:203], in0=Gt[:, 0:SM, 102:203],
                        in1=wv[:, 0:SM, 1:2].broadcast_to([P, SM, 101]))

                def agg_reduce(li, w):
                    last = li == 4
                    SM = SMw[w]
                    s = state.pop(w)
                    st, Gt = s["st"], s["Gt"]
                    apsum = pspool.tile([P, SCOLS], F32, tag="apsum")
                    for j in range(SM):
                        nc.tensor.matmul(
                            out=apsum[:], lhsT=st[:, j * P:(j + 1) * P],
                            rhs=Gt[:, j, 0:SCOLS],
                            start=(j == 0), stop=(j == SM - 1))
                    sc = pool.tile([P, 2], F32, tag="sc")
                    nc.vector.tensor_scalar_add(
                        out=sc[:], in0=apsum[:, 100:SCOLS:102], scalar1=1e-30)
                    rc = pool.tile([P, 2], F32, tag="rc")
                    nc.vector.reciprocal(out=rc[:], in_=sc[:])
                    stb = pool.tile([P, SCOLS], BF16, tag="stb")
                    if has_bias:
                        sg = pool.tile([P, SCOLS], F32, tag="sg")
                        nc.vector.tensor_scalar_mul(
                            out=sg[:, 0:101], in0=apsum[:, 0:101], scalar1=rc[:, 0:1])
                        nc.vector.tensor_scalar_mul(
                            out=sg[:, 101:SCOLS], in0=apsum[:, 101:SCOLS],
                            scalar1=rc[:, 1:2])
                        nc.vector.tensor_add(out=sg[:], in0=sg[:], in1=brts[li][:])
                        nc.scalar.activation(out=stb[:], in_=sg[:], func=AF.Relu)
                    else:
                        nc.scalar.activation(out=stb[:, 0:101], in_=apsum[:, 0:101],
                                             func=AF.Relu, scale=rc[:, 0:1])
                        nc.scalar.activation(out=stb[:, 101:SCOLS],
                                             in_=apsum[:, 101:SCOLS],
                                             func=AF.Relu, scale=rc[:, 1:2])
                    if not last:
                        nxt = li % 2
                        tp = ppool.tile([P, 2 * P], BF16, tag="tp")
                        nc.tensor.transpose(out=tp[:, 0:P], in_=stb[:, 0:P],
                                            identity=ident[:])
                        nc.tensor.transpose(out=tp[0:SCOLS - P, P:2 * P],
                                            in_=stb[:, P:SCOLS], identity=ident[:])
                        c0 = w * P
                        nc.scalar.activation(out=xTA[nxt][:, c0:c0 + P],
                                             in_=tp[:, 0:P], func=AF.Copy)
                        nc.scalar.activation(out=xTB[nxt][:, c0:c0 + P],
                                             in_=tp[0:SCOLS - P, P:2 * P],
                                             func=AF.Copy)
                    else:
                        pmt = pool.tile([P, G], BF16, tag="pmt")
                        nc.scalar.dma_start(out=pmt[:], in_=pmaskb[w])
                        nc.tensor.matmul(out=ppA[:], lhsT=stb[:, 0:P], rhs=pmt[:],
                                         start=(w == 0), stop=(w == NW - 1))
                        nc.tensor.matmul(out=ppB[:], lhsT=stb[:, P:SCOLS],
                                         rhs=pmt[:],
                                         start=(w == 0), stop=(w == NW - 1))

                # ---- schedule: node0; per layer chunked CC + 3-stage pipeline ----
                for t in range(NT):
                    node_tile(0, t)
                for li in range(5):
                    Tf = TfullA if li % 2 == 0 else TfullB
                    nc.gpsimd.collective_compute(
                        "AllGather", ALU.bypass, replica_groups=rg,
                        ins=[Tshard[0:RHALF, :]], outs=[Tf[0:CHA, :]])
                    nc.gpsimd.collective_compute(
                        "AllGather", ALU.bypass, replica_groups=rg,
                        ins=[Tshard[RHALF:SHARD_PAD, :]],
                        outs=[Tf[CHA:NFULL_PAD, :]])
                    for w in range(NW + 2):
                        if w < NW:
                            agg_load(li, w, Tf)
                        if 1 <= w <= NW:
                            agg_score(li, w - 1)
                        if w >= 2:
                            agg_reduce(li, w - 2)
                            if li < 4:
                                node_tile(li + 1, w - 2)

                # ---- pool tail: AllReduce + MLP ----
                cpA = pool.tile([P, G], F32, tag="cpA")
                nc.vector.tensor_copy(out=cpA[:], in_=ppA[:])
                cpB = pool.tile([SCOLS - P, G], F32, tag="cpB")
                nc.vector.tensor_copy(out=cpB[:], in_=ppB[:])
                nc.sync.dma_start(out=cc2_in[0:P, :], in_=cpA[:])
                nc.sync.dma_start(out=cc2_in[P:SCOLS, :], in_=cpB[:])
              with tc.tile_pool(name="ps2", bufs=1, space="PSUM") as pspool:
                tc.strict_bb_all_engine_barrier()
                nc.gpsimd.collective_compute(
                    "AllReduce", ALU.add, replica_groups=rg,
                    ins=[cc2_in[:]], outs=[cc2_out[:]])
                tc.strict_bb_all_engine_barrier()
                plA = pool.tile([P, G], F32, tag="plA")
                nc.sync.dma_start(out=plA[:], in_=cc2_out[0:P, :])
                plB = pool.tile([SCOLS - P, G], F32, tag="plB")
                nc.sync.dma_start(out=plB[:], in_=cc2_out[P:SCOLS, :])
                w1a = pool.tile([P, 100], F32, tag="w1a")
                nc.sync.dma_start(out=w1a[:], in_=mlw1a_in[:])
                w1b = pool.tile([SCOLS - P, 100], F32, tag="w1b")
                nc.sync.dma_start(out=w1b[:], in_=mlw1b_in[0:SCOLS - P, :])
                w2t = pool.tile([100, 100], F32, tag="w2t")
                nc.sync.dma_start(out=w2t[:], in_=mlw2_in[:])
                w3t = pool.tile([100, 29], F32, tag="w3t")
                nc.sync.dma_start(out=w3t[:], in_=mlw3_in[:])
                if has_bias:
                    b1 = pool.tile([P, 1], F32, tag="b1")
                    nc.sync.dma_start(out=b1[:], in_=mlpb[0])
                    b2 = pool.tile([P, 1], F32, tag="b2")
                    nc.sync.dma_start(out=b2[:], in_=mlpb[1])
                    b3 = pool.tile([P, 1], F32, tag="b3")
                    nc.sync.dma_start(out=b3[:], in_=mlpb[2])
                y1p = pspool.tile([100, G], F32, tag="y1p")
                nc.tensor.matmul(out=y1p[:], lhsT=w1a[:], rhs=plA[:],
                                 start=True, stop=False)
                nc.tensor.matmul(out=y1p[:], lhsT=w1b[:], rhs=plB[:],
                                 start=False, stop=True)
                y1 = pool.tile([100, G], F32, tag="y1")
                nc.scalar.activation(out=y1[:], in_=y1p[:], func=AF.Relu,
                                     bias=b1[0:100, :] if has_bias else 0.0)
                y2p = pspool.tile([100, G], F32, tag="y2p")
                nc.tensor.matmul(out=y2p[:], lhsT=w2t[:], rhs=y1[:],
                                 start=True, stop=True)
                y2 = pool.tile([100, G], F32, tag="y2")
                nc.scalar.activation(out=y2[:], in_=y2p[:], func=AF.Relu,
                                     bias=b2[0:100, :] if has_bias else 0.0)
                y3p = pspool.tile([29, G], F32, tag="y3p")
                nc.tensor.matmul(out=y3p[:], lhsT=w3t[:], rhs=y2[:],
                                 start=True, stop=True)
                y3 = pool.tile([29, G], F32, tag="y3")
                nc.scalar.activation(out=y3[:], in_=y3p[:], func=AF.Identity,
                                     bias=b3[0:29, :] if has_bias else 0.0)
                nc.sync.dma_start(out=out[:], in_=y3[:])

    nc.finalize()
    return nc


def make_in_maps(per_core, shared):
    return [{**pc, **shared} for pc in per_core]


# ---------------- runner (device-resident SPMD invoke) ----------------
import jax
from jax.sharding import Mesh, PartitionSpec, NamedSharding
from jax.experimental.shard_map import shard_map
from concourse import bass2jax
from concourse.bass2jax import _bass_exec_p, install_neuronx_cc_hook, partition_id_tensor


class SpmdRunner:
    def __init__(self, nc, n_cores=8):
        install_neuronx_cc_hook()
        self.nc = nc
        self.n_cores = n_cores
        partition_name = nc.partition_id_tensor.name if nc.partition_id_tensor else None
        in_names, out_names, out_avals, zero_outs = [], [], [], []
        for alloc in nc.m.functions[0].allocations:
            if not isinstance(alloc, mybir.MemoryLocationSet):
                continue
            name = alloc.memorylocations[0].name
            if alloc.kind == "ExternalInput":
                if name != partition_name and name != (nc.dbg_addr.name if nc.dbg_addr else None):
                    in_names.append(name)
            elif alloc.kind == "ExternalOutput":
                out_names.append(name)
                shape = tuple(alloc.tensor_shape)
                dtype = mybir.dt.np(alloc.dtype)
                out_avals.append(jax.core.ShapedArray(shape, dtype))
                zero_outs.append(np.zeros(shape, dtype))
        self.in_names, self.out_names = in_names, out_names
        self.out_avals, self.zero_outs = out_avals, zero_outs
        n_params, n_outs = len(in_names), len(out_names)
        self.n_params = n_params
        all_in_names = list(in_names) + list(out_names)
        if nc.dbg_addr is not None:
            all_in_names.append(nc.dbg_addr.name)
        if partition_name is not None:
            all_in_names.append(partition_name)
        self.has_dbg = nc.dbg_addr is not None

        def _body(*args):
            operands = list(args)
            if self.has_dbg:
                operands.append(jax.numpy.zeros((1, 2), jax.numpy.uint32))
            if partition_name is not None:
                operands.append(partition_id_tensor())
            outs = _bass_exec_p.bind(
                *operands,
                out_avals=tuple(out_avals),
                in_names=tuple(all_in_names),
                out_names=tuple(out_names),
                lowering_input_output_aliases=(),
                sim_require_finite=False,
                sim_require_nnan=False,
                nc=nc,
            )
            return tuple(outs)

        devices = jax.devices()[:n_cores]
        self.mesh = Mesh(np.asarray(devices), ("core",))
        in_specs = (PartitionSpec("core"),) * (n_params + n_outs)
        out_specs = (PartitionSpec("core"),) * n_outs
        donate = tuple(range(n_params, n_params + n_outs))
        self.sharded = jax.jit(
            shard_map(_body, mesh=self.mesh, in_specs=in_specs,
                      out_specs=out_specs, check_rep=False),
            donate_argnums=donate, keep_unused=True,
        )
        self.sharding = NamedSharding(self.mesh, PartitionSpec("core"))
        self.dev_in = None

    def stage_inputs(self, in_maps):
        per_core = [[np.asarray(m[n]) for n in self.in_names] for m in in_maps]
        concat_in = [
            np.concatenate([per_core[c][i] for c in range(self.n_cores)], axis=0)
            for i in range(self.n_params)
        ]
        self.dev_in = [jax.device_put(a, self.sharding) for a in concat_in]
        for a in self.dev_in:
            a.block_until_ready()

    def __call__(self):
        concat_zeros = [
            jax.device_put(
                np.zeros((self.n_cores * z.shape[0], *z.shape[1:]), z.dtype),
                self.sharding)
            for z in self.zero_outs
        ]
        out = self.sharded(*self.dev_in, *concat_zeros)
        for o in out:
            o.block_until_ready()
        return out

    def results(self, out):
        return [
            {
                name: np.asarray(out[i]).reshape(self.n_cores, *self.out_avals[i].shape)[c]
                for i, name in enumerate(self.out_names)
            }
            for c in range(self.n_cores)
        ]


# ---------------- entry point ----------------
_CACHE = {}


def _get_runner(meta):
    key = (meta["Q1w"], meta["Q2w"], meta["N1w"], meta["N2w"], meta["has_bias"])
    if key not in _CACHE:
        nc = build_nc(list(meta["Q1w"]), list(meta["Q2w"]),
                      list(meta["N1w"]), list(meta["N2w"]), meta["has_bias"])
        _CACHE[key] = SpmdRunner(nc, NCORE)
    return _CACHE[key]


def kernel(**inputs):
    x = np.asarray(inputs["x"], np.float32)
    edge_index = np.asarray(inputs["edge_index"])
    batch = np.asarray(inputs["batch"])
    Ws = [np.asarray(inputs[f"W{i+1}"], np.float32) for i in range(5)]
    asrcs = [np.asarray(inputs[f"asrc{i+1}"], np.float32) for i in range(5)]
    adsts = [np.asarray(inputs[f"adst{i+1}"], np.float32) for i in range(5)]
    bcs = [np.asarray(inputs[f"bc{i+1}"], np.float32) for i in range(5)]
    lws = [np.asarray(inputs[f"lw{i+1}"], np.float32) for i in range(3)]
    lbs = [np.asarray(inputs[f"lb{i+1}"], np.float32) for i in range(3)]
    per_core, shared, meta = prep(x, edge_index, batch, Ws, asrcs, adsts, bcs, lws, lbs)
    r = _get_runner(meta)
    r.stage_inputs(make_in_maps(per_core, shared))
    out = r()
    y3 = r.results(out)[0]["out"]       # [29, 256]
    return np.ascontiguousarray(y3.T)   # [256, 29]
